# revision 2
# baseline (speedup 1.0000x reference)
"""LinkPredictor Bass kernel for 8 Trainium2 NeuronCores.

Model: scores[e] = W2 . relu([emb[src[e]] ; emb[dst[e]]] @ W1 + b1) + b2
  embeddings [100000, 256] f32, edge_index [2, 500000] int64,
  W1 [512, 256], b1 [256], W2 [256, 1], b2 [1].

Strategy (hardcoded, self-contained):
  * Shard edges across the 8 cores (62500 each); replicate embeddings
    (cast to bf16 on host) and the tiny MLP weights.
  * Gather the two embedding rows per edge with SWDGE dma_gather in
    transpose mode, which lands the rows channel-major in SBUF --
    exactly the layout the tensor engine needs for the MLP matmuls.
  * dma_gather indices are int16, so the node table is viewed as 4
    sub-tables of 32768 rows; each core's edges are bucketed on host by
    (src_subtable, dst_subtable) into 16 runs, padded to multiples of
    128.  Scores come back in run order and are un-permuted on host.
  * Layer 1 runs as 8 bf16 matmuls (K=4x128, M=2x128, N<=512) per
    512-edge segment accumulating in PSUM; relu+bias on the scalar
    engine; layer 2 is a 2-matmul dot with W2; bias b2 added during the
    PSUM->SBUF copy; scores DMA'd back to DRAM.
"""

import os
import numpy as np
import ml_dtypes

N_NODES = 100000
IN_CH = 256
HIDDEN = 256
N_EDGES = 500000
N_CORES = 8
EDGES_PER_CORE = N_EDGES // N_CORES
SUB = 32768          # sub-table rows addressable by int16 gather indices
NSUB = 4             # ceil(100000 / 32768)
GCHUNK = 512         # max edges per dma_gather call (>=1024 kills the device)
SEG = 512            # edges per matmul segment (PSUM bank = 512 fp32)

_PROG_CACHE = {}


def _host_prep(edge_index):
    """Bucket each core's edges into 16 (src_sub, dst_sub) runs.

    Returns per-core index layouts plus the shared (max-over-cores,
    padded) run sizes that define the SPMD program shape.
    """
    src = np.asarray(edge_index[0]).astype(np.int64)
    dst = np.asarray(edge_index[1]).astype(np.int64)

    per_core = []
    counts = np.zeros((N_CORES, NSUB * NSUB), dtype=np.int64)
    for c in range(N_CORES):
        lo, hi = c * EDGES_PER_CORE, (c + 1) * EDGES_PER_CORE
        s, d = src[lo:hi], dst[lo:hi]
        bucket = (s // SUB) * NSUB + (d // SUB)
        order = np.argsort(bucket, kind="stable")
        per_core.append((s, d, bucket, order))
        counts[c] = np.bincount(bucket, minlength=NSUB * NSUB)

    run_sizes = []
    for r in range(NSUB * NSUB):
        n = int(counts[:, r].max())
        run_sizes.append(((n + 127) // 128) * 128 if n > 0 else 0)

    e_pad = int(sum(run_sizes))
    totcols = 2 * e_pad // 16  # int16 idx columns (src+dst, 16-wrapped)

    core_data = []
    for c in range(N_CORES):
        s, d, bucket, order = per_core[c]
        idx_cols = np.zeros((16, totcols), dtype=np.int16)
        # positions[r] = (edge positions within this core's slice, run offset)
        positions = []
        col_off = 0
        edge_off = 0
        sorted_bucket = bucket[order]
        run_starts = np.searchsorted(sorted_bucket, np.arange(NSUB * NSUB))
        run_ends = np.searchsorted(sorted_bucket, np.arange(NSUB * NSUB), "right")
        for r in range(NSUB * NSUB):
            L = run_sizes[r]
            if L == 0:
                positions.append((np.empty(0, np.int64), 0))
                continue
            sel = order[run_starts[r]:run_ends[r]]
            ssub, dsub = r // NSUB, r % NSUB
            s_loc = np.zeros(L, dtype=np.int16)
            d_loc = np.zeros(L, dtype=np.int16)
            s_loc[: len(sel)] = (s[sel] - ssub * SUB).astype(np.int16)
            d_loc[: len(sel)] = (d[sel] - dsub * SUB).astype(np.int16)
            ncol = L // 16
            idx_cols[:, col_off:col_off + ncol] = s_loc.reshape(ncol, 16).T
            idx_cols[:, col_off + ncol:col_off + 2 * ncol] = d_loc.reshape(ncol, 16).T
            positions.append((sel, edge_off))
            col_off += 2 * ncol
            edge_off += L
        idx_full = np.tile(idx_cols, (8, 1))  # replicate to 128 partitions
        core_data.append((idx_full, positions))

    return run_sizes, e_pad, totcols, core_data


def _build_program(run_sizes, e_pad, totcols):
    import concourse.bacc as bacc
    import concourse.bass as bass
    import concourse.mybir as mybir
    import concourse.tile as tile

    nc = bacc.Bacc("TRN2", target_bir_lowering=False, debug=False,
                   num_devices=N_CORES)

    emb_t = nc.dram_tensor("emb", [N_NODES, IN_CH], mybir.dt.bfloat16,
                           kind="ExternalInput")
    idx_t = nc.dram_tensor("idx", [128, totcols], mybir.dt.int16,
                           kind="ExternalInput")
    w1_t = nc.dram_tensor("w1t", [128, 8 * 128], mybir.dt.bfloat16,
                          kind="ExternalInput")
    w2_t = nc.dram_tensor("w2t", [128, 2], mybir.dt.bfloat16,
                          kind="ExternalInput")
    b1_t = nc.dram_tensor("b1t", [128, 2], mybir.dt.float32,
                          kind="ExternalInput")
    b2_t = nc.dram_tensor("b2t", [1, 1], mybir.dt.float32,
                          kind="ExternalInput")
    out_t = nc.dram_tensor("scores", [1, e_pad], mybir.dt.float32,
                           kind="ExternalOutput")

    relu = mybir.ActivationFunctionType.Relu
    ident = mybir.ActivationFunctionType.Identity

    with tile.TileContext(nc) as tc:
        with (
            tc.tile_pool(name="const", bufs=1) as cpool,
            tc.tile_pool(name="gather", bufs=3) as gpool,
            tc.tile_pool(name="act", bufs=4) as apool,
            tc.tile_pool(name="psum", bufs=4, space="PSUM") as ppool,
            tc.tile_pool(name="spsum", bufs=2, space="PSUM") as spool,
        ):
            idx_sb = cpool.tile([128, totcols], mybir.dt.int16, tag="idx")
            w1_sb = cpool.tile([128, 8 * 128], mybir.dt.bfloat16, tag="w1")
            w2_sb = cpool.tile([128, 2], mybir.dt.bfloat16, tag="w2")
            b1_sb = cpool.tile([128, 2], mybir.dt.float32, tag="b1")
            b2_sb = cpool.tile([1, 1], mybir.dt.float32, tag="b2")
            nc.sync.dma_start(out=idx_sb[:], in_=idx_t[:])
            nc.sync.dma_start(out=w1_sb[:], in_=w1_t[:])
            nc.sync.dma_start(out=w2_sb[:], in_=w2_t[:])
            nc.sync.dma_start(out=b1_sb[:], in_=b1_t[:])
            nc.sync.dma_start(out=b2_sb[:], in_=b2_t[:])

            col_off = 0
            edge_off = 0
            for r in range(NSUB * NSUB):
                L = run_sizes[r]
                if L == 0:
                    continue
                ssub, dsub = r // NSUB, r % NSUB
                s_rows = min(SUB, N_NODES - ssub * SUB)
                d_rows = min(SUB, N_NODES - dsub * SUB)
                src_tab = emb_t[ssub * SUB: ssub * SUB + s_rows]
                dst_tab = emb_t[dsub * SUB: dsub * SUB + d_rows]
                ncol = L // 16
                src_cols = col_off
                dst_cols = col_off + ncol

                for c0 in range(0, L, GCHUNK):
                    Lc = min(GCHUNK, L - c0)
                    src_g = gpool.tile([128, 2, Lc], mybir.dt.bfloat16,
                                       tag="srcg")
                    dst_g = gpool.tile([128, 2, Lc], mybir.dt.bfloat16,
                                       tag="dstg")
                    nc.gpsimd.dma_gather(
                        src_g[:], src_tab,
                        idx_sb[:, src_cols + c0 // 16:
                               src_cols + (c0 + Lc) // 16],
                        Lc, Lc, IN_CH, transpose=True)
                    nc.gpsimd.dma_gather(
                        dst_g[:], dst_tab,
                        idx_sb[:, dst_cols + c0 // 16:
                               dst_cols + (c0 + Lc) // 16],
                        Lc, Lc, IN_CH, transpose=True)

                    for g0 in range(0, Lc, SEG):
                        N = min(SEG, Lc - g0)
                        relu_sb = []
                        for m in range(2):
                            h_ps = ppool.tile([128, SEG], mybir.dt.float32,
                                              tag="hps")
                            mm = 0
                            for tab, g_tile in ((0, src_g), (1, dst_g)):
                                for kk in range(2):
                                    blk = (tab * 2 + kk) * 2 + m
                                    nc.tensor.matmul(
                                        h_ps[:, :N],
                                        lhsT=w1_sb[:, blk * 128:(blk + 1) * 128],
                                        rhs=g_tile[:, kk, g0:g0 + N],
                                        start=(mm == 0), stop=(mm == 3))
                                    mm += 1
                            r_sb = apool.tile([128, SEG], mybir.dt.bfloat16,
                                              tag=f"relu{m}")
                            nc.scalar.activation(r_sb[:, :N], h_ps[:, :N],
                                                 relu, bias=b1_sb[:, m:m + 1])
                            relu_sb.append(r_sb)

                        s_ps = spool.tile([1, SEG], mybir.dt.float32,
                                          tag="sps")
                        for m in range(2):
                            nc.tensor.matmul(
                                s_ps[:1, :N],
                                lhsT=w2_sb[:, m:m + 1],
                                rhs=relu_sb[m][:, :N],
                                start=(m == 0), stop=(m == 1))
                        s_sb = apool.tile([1, SEG], mybir.dt.float32,
                                          tag="ssb")
                        nc.scalar.activation(s_sb[:1, :N], s_ps[:1, :N],
                                             ident, bias=b2_sb[:1, :1])
                        off = edge_off + c0 + g0
                        nc.sync.dma_start(out=out_t[0:1, off:off + N],
                                          in_=s_sb[:1, :N])
                col_off += 2 * ncol
                edge_off += L

    nc.compile()
    return nc


def kernel(embeddings, edge_index, W1, b1, W2, b2):
    from concourse.bass_utils import run_bass_kernel_spmd

    embeddings = np.asarray(embeddings, dtype=np.float32)
    W1 = np.asarray(W1, dtype=np.float32)
    b1 = np.asarray(b1, dtype=np.float32)
    W2 = np.asarray(W2, dtype=np.float32)
    b2 = np.asarray(b2, dtype=np.float32)

    run_sizes, e_pad, totcols, core_data = _host_prep(edge_index)

    key = (tuple(run_sizes), e_pad, totcols)
    if key not in _PROG_CACHE:
        _PROG_CACHE[key] = _build_program(run_sizes, e_pad, totcols)
    nc = _PROG_CACHE[key]

    emb_bf16 = embeddings.astype(ml_dtypes.bfloat16)
    # w1t[p, blk*128+j] = W1[kg*128+p, m*128+j], blk = kg*2+m
    w1t = np.empty((128, 8 * 128), dtype=ml_dtypes.bfloat16)
    for kg in range(4):
        for m in range(2):
            blk = kg * 2 + m
            w1t[:, blk * 128:(blk + 1) * 128] = W1[
                kg * 128:(kg + 1) * 128, m * 128:(m + 1) * 128]
    w2t = np.empty((128, 2), dtype=ml_dtypes.bfloat16)
    w2t[:, 0] = W2[:128, 0]
    w2t[:, 1] = W2[128:, 0]
    b1t = np.ascontiguousarray(b1.reshape(2, 128).T, dtype=np.float32)
    b2t = b2.reshape(1, 1)

    in_maps = []
    for c in range(N_CORES):
        idx_full, _ = core_data[c]
        in_maps.append({
            "emb": emb_bf16,
            "idx": idx_full,
            "w1t": w1t,
            "w2t": w2t,
            "b1t": b1t,
            "b2t": b2t,
        })

    trace = bool(int(os.environ.get("LINKPRED_TRACE", "0")))
    res = run_bass_kernel_spmd(nc, in_maps, core_ids=list(range(N_CORES)),
                               trace=trace)
    kernel._last_results = res

    scores = np.empty(N_EDGES, dtype=np.float32)
    for c in range(N_CORES):
        _, positions = core_data[c]
        core_out = res.results[c]["scores"].reshape(-1)
        base = c * EDGES_PER_CORE
        for r in range(NSUB * NSUB):
            sel, edge_off = positions[r]
            if len(sel):
                scores[base + sel] = core_out[edge_off:edge_off + len(sel)]
    return scores


# revision 7
# speedup vs baseline: 2.2851x; 2.2851x over previous
"""LinkPredictor Bass kernel for 8 Trainium2 NeuronCores.

Model: scores[e] = W2 . relu([emb[src[e]] ; emb[dst[e]]] @ W1 + b1) + b2
  embeddings [100000, 256] f32, edge_index [2, 500000] int64,
  W1 [512, 256], b1 [256], W2 [256, 1], b2 [1].

Strategy (hardcoded, self-contained):
  * Shard edges across the 8 cores (62500 each); replicate embeddings
    (cast to bf16 on host) and the tiny MLP weights.
  * Gather the two embedding rows per edge with SWDGE dma_gather in
    transpose mode, which lands the rows channel-major in SBUF --
    exactly the layout the tensor engine needs for the MLP matmuls.
  * dma_gather indices are int16, so the node table is viewed as 4
    sub-tables of 32768 rows; each core's edges are bucketed on host by
    (src_subtable, dst_subtable) into 16 runs, padded to multiples of
    128.  Scores come back in run order and are un-permuted on host.
  * Layer 1 runs as 8 bf16 matmuls (K=4x128, M=2x128, N<=512) per
    512-edge segment accumulating in PSUM; relu+bias on the scalar
    engine; layer 2 is a 2-matmul dot with W2; bias b2 added during the
    PSUM->SBUF copy; scores DMA'd back to DRAM.
"""

import os
import numpy as np
import ml_dtypes

N_NODES = 100000
IN_CH = 256
HIDDEN = 256
N_EDGES = 500000
N_CORES = 8
EDGES_PER_CORE = N_EDGES // N_CORES
SUB = 32768          # sub-table rows addressable by int16 gather indices
NSUB = 4             # ceil(100000 / 32768)
GCHUNK = 512         # max edges per dma_gather call (>=1024 kills the device)
SEG = 512            # edges per matmul segment (PSUM bank = 512 fp32)

_PROG_CACHE = {}


def _host_prep(edge_index):
    """Bucket each core's edges into 16 (src_sub, dst_sub) runs.

    Returns per-core index layouts plus the shared (max-over-cores,
    padded) run sizes that define the SPMD program shape.
    """
    src = np.asarray(edge_index[0]).astype(np.int64)
    dst = np.asarray(edge_index[1]).astype(np.int64)

    per_core = []
    counts = np.zeros((N_CORES, NSUB * NSUB), dtype=np.int64)
    for c in range(N_CORES):
        lo, hi = c * EDGES_PER_CORE, (c + 1) * EDGES_PER_CORE
        s, d = src[lo:hi], dst[lo:hi]
        bucket = (s // SUB) * NSUB + (d // SUB)
        order = np.argsort(bucket, kind="stable")
        per_core.append((s, d, bucket, order))
        counts[c] = np.bincount(bucket, minlength=NSUB * NSUB)

    run_sizes = []
    for r in range(NSUB * NSUB):
        n = int(counts[:, r].max())
        run_sizes.append(((n + 127) // 128) * 128 if n > 0 else 0)

    e_pad = int(sum(run_sizes))
    totcols = 2 * e_pad // 16  # int16 idx columns (src+dst, 16-wrapped)

    core_data = []
    for c in range(N_CORES):
        s, d, bucket, order = per_core[c]
        idx_cols = np.zeros((16, totcols), dtype=np.int16)
        # positions[r] = (edge positions within this core's slice, run offset)
        positions = []
        col_off = 0
        edge_off = 0
        sorted_bucket = bucket[order]
        run_starts = np.searchsorted(sorted_bucket, np.arange(NSUB * NSUB))
        run_ends = np.searchsorted(sorted_bucket, np.arange(NSUB * NSUB), "right")
        for r in range(NSUB * NSUB):
            L = run_sizes[r]
            if L == 0:
                positions.append((np.empty(0, np.int64), 0))
                continue
            sel = order[run_starts[r]:run_ends[r]]
            ssub, dsub = r // NSUB, r % NSUB
            s_loc = np.zeros(L, dtype=np.int16)
            d_loc = np.zeros(L, dtype=np.int16)
            s_loc[: len(sel)] = (s[sel] - ssub * SUB).astype(np.int16)
            d_loc[: len(sel)] = (d[sel] - dsub * SUB).astype(np.int16)
            ncol = L // 16
            idx_cols[:, col_off:col_off + ncol] = s_loc.reshape(ncol, 16).T
            idx_cols[:, col_off + ncol:col_off + 2 * ncol] = d_loc.reshape(ncol, 16).T
            positions.append((sel, edge_off))
            col_off += 2 * ncol
            edge_off += L
        idx_full = np.tile(idx_cols, (8, 1))  # replicate to 128 partitions
        core_data.append((idx_full, positions))

    return run_sizes, e_pad, totcols, core_data


def _build_program(run_sizes, e_pad, totcols):
    import concourse.bacc as bacc
    import concourse.bass as bass
    import concourse.mybir as mybir
    import concourse.tile as tile

    nc = bacc.Bacc("TRN2", target_bir_lowering=False, debug=False,
                   num_devices=N_CORES, num_swdge_queues=4)

    emb_t = nc.dram_tensor("emb", [N_NODES, IN_CH], mybir.dt.bfloat16,
                           kind="ExternalInput")
    idx_t = nc.dram_tensor("idx", [128, totcols], mybir.dt.int16,
                           kind="ExternalInput")
    w1_t = nc.dram_tensor("w1t", [128, 8 * 128], mybir.dt.bfloat16,
                          kind="ExternalInput")
    w2_t = nc.dram_tensor("w2t", [128, 2], mybir.dt.bfloat16,
                          kind="ExternalInput")
    b1_t = nc.dram_tensor("b1t", [128, 2], mybir.dt.float32,
                          kind="ExternalInput")
    b2_t = nc.dram_tensor("b2t", [1, 1], mybir.dt.float32,
                          kind="ExternalInput")
    out_t = nc.dram_tensor("scores", [1, e_pad], mybir.dt.float32,
                           kind="ExternalOutput")

    relu = mybir.ActivationFunctionType.Relu
    _build_program._qn = 0

    with tile.TileContext(nc) as tc:
        with (
            tc.tile_pool(name="const", bufs=1) as cpool,
            tc.tile_pool(name="gather", bufs=3) as gpool,
            tc.tile_pool(name="act", bufs=4) as apool,
            tc.tile_pool(name="psum", bufs=4, space="PSUM") as ppool,
            tc.tile_pool(name="spsum", bufs=2, space="PSUM") as spool,
        ):
            idx_sb = cpool.tile([128, totcols], mybir.dt.int16, tag="idx")
            w1_sb = cpool.tile([128, 8 * 128], mybir.dt.bfloat16, tag="w1")
            w2_sb = cpool.tile([128, 2], mybir.dt.bfloat16, tag="w2")
            b1_sb = cpool.tile([128, 2], mybir.dt.float32, tag="b1")
            b2_sb = cpool.tile([1, 1], mybir.dt.float32, tag="b2")
            nc.sync.dma_start(out=idx_sb[:], in_=idx_t[:])
            nc.sync.dma_start(out=w1_sb[:], in_=w1_t[:])
            nc.sync.dma_start(out=w2_sb[:], in_=w2_t[:])
            nc.sync.dma_start(out=b1_sb[:], in_=b1_t[:])
            nc.sync.dma_start(out=b2_sb[:], in_=b2_t[:])

            col_off = 0
            edge_off = 0
            for r in range(NSUB * NSUB):
                L = run_sizes[r]
                if L == 0:
                    continue
                ssub, dsub = r // NSUB, r % NSUB
                s_rows = min(SUB, N_NODES - ssub * SUB)
                d_rows = min(SUB, N_NODES - dsub * SUB)
                src_tab = emb_t[ssub * SUB: ssub * SUB + s_rows]
                dst_tab = emb_t[dsub * SUB: dsub * SUB + d_rows]
                ncol = L // 16
                src_cols = col_off
                dst_cols = col_off + ncol

                for c0 in range(0, L, GCHUNK):
                    Lc = min(GCHUNK, L - c0)
                    src_g = gpool.tile([128, 2, Lc], mybir.dt.bfloat16,
                                       tag="srcg")
                    dst_g = gpool.tile([128, 2, Lc], mybir.dt.bfloat16,
                                       tag="dstg")
                    nc.gpsimd.dma_gather(
                        src_g[:], src_tab,
                        idx_sb[:, src_cols + c0 // 16:
                               src_cols + (c0 + Lc) // 16],
                        Lc, Lc, IN_CH, transpose=True,
                        queue_num=_build_program._qn % 4)
                    _build_program._qn += 1
                    nc.gpsimd.dma_gather(
                        dst_g[:], dst_tab,
                        idx_sb[:, dst_cols + c0 // 16:
                               dst_cols + (c0 + Lc) // 16],
                        Lc, Lc, IN_CH, transpose=True,
                        queue_num=_build_program._qn % 4)
                    _build_program._qn += 1

                    for g0 in range(0, Lc, SEG):
                        N = min(SEG, Lc - g0)
                        relu_sb = []
                        for m in range(2):
                            h_ps = ppool.tile([128, SEG], mybir.dt.float32,
                                              tag="hps")
                            mm = 0
                            for tab, g_tile in ((0, src_g), (1, dst_g)):
                                for kk in range(2):
                                    blk = (tab * 2 + kk) * 2 + m
                                    nc.tensor.matmul(
                                        h_ps[:, :N],
                                        lhsT=w1_sb[:, blk * 128:(blk + 1) * 128],
                                        rhs=g_tile[:, kk, g0:g0 + N],
                                        start=(mm == 0), stop=(mm == 3))
                                    mm += 1
                            r_sb = apool.tile([128, SEG], mybir.dt.bfloat16,
                                              tag=f"relu{m}")
                            if m == 0:
                                nc.scalar.activation(r_sb[:, :N], h_ps[:, :N],
                                                     relu,
                                                     bias=b1_sb[:, m:m + 1])
                            else:
                                # balance: second half on the vector engine
                                nc.vector.tensor_scalar(
                                    out=r_sb[:, :N], in0=h_ps[:, :N],
                                    scalar1=b1_sb[:, m:m + 1], scalar2=0.0,
                                    op0=mybir.AluOpType.add,
                                    op1=mybir.AluOpType.max)
                            relu_sb.append(r_sb)

                        s_ps = spool.tile([1, SEG], mybir.dt.float32,
                                          tag="sps")
                        for m in range(2):
                            nc.tensor.matmul(
                                s_ps[:1, :N],
                                lhsT=w2_sb[:, m:m + 1],
                                rhs=relu_sb[m][:, :N],
                                start=(m == 0), stop=(m == 1))
                        s_sb = apool.tile([1, SEG], mybir.dt.float32,
                                          tag="ssb")
                        nc.vector.tensor_scalar_add(
                            out=s_sb[:1, :N], in0=s_ps[:1, :N],
                            scalar1=b2_sb[:1, :1])
                        off = edge_off + c0 + g0
                        nc.sync.dma_start(out=out_t[0:1, off:off + N],
                                          in_=s_sb[:1, :N])
                col_off += 2 * ncol
                edge_off += L

    nc.compile()
    return nc


def kernel(embeddings, edge_index, W1, b1, W2, b2):
    from concourse.bass_utils import run_bass_kernel_spmd

    embeddings = np.asarray(embeddings, dtype=np.float32)
    W1 = np.asarray(W1, dtype=np.float32)
    b1 = np.asarray(b1, dtype=np.float32)
    W2 = np.asarray(W2, dtype=np.float32)
    b2 = np.asarray(b2, dtype=np.float32)

    run_sizes, e_pad, totcols, core_data = _host_prep(edge_index)

    key = (tuple(run_sizes), e_pad, totcols)
    if key not in _PROG_CACHE:
        _PROG_CACHE[key] = _build_program(run_sizes, e_pad, totcols)
    nc = _PROG_CACHE[key]

    emb_bf16 = embeddings.astype(ml_dtypes.bfloat16)
    # w1t[p, blk*128+j] = W1[kg*128+p, m*128+j], blk = kg*2+m
    w1t = np.empty((128, 8 * 128), dtype=ml_dtypes.bfloat16)
    for kg in range(4):
        for m in range(2):
            blk = kg * 2 + m
            w1t[:, blk * 128:(blk + 1) * 128] = W1[
                kg * 128:(kg + 1) * 128, m * 128:(m + 1) * 128]
    w2t = np.empty((128, 2), dtype=ml_dtypes.bfloat16)
    w2t[:, 0] = W2[:128, 0]
    w2t[:, 1] = W2[128:, 0]
    b1t = np.ascontiguousarray(b1.reshape(2, 128).T, dtype=np.float32)
    b2t = b2.reshape(1, 1)

    in_maps = []
    for c in range(N_CORES):
        idx_full, _ = core_data[c]
        in_maps.append({
            "emb": emb_bf16,
            "idx": idx_full,
            "w1t": w1t,
            "w2t": w2t,
            "b1t": b1t,
            "b2t": b2t,
        })

    trace = bool(int(os.environ.get("LINKPRED_TRACE", "0")))
    res = run_bass_kernel_spmd(nc, in_maps, core_ids=list(range(N_CORES)),
                               trace=trace)
    kernel._last_results = res

    scores = np.empty(N_EDGES, dtype=np.float32)
    for c in range(N_CORES):
        _, positions = core_data[c]
        core_out = res.results[c]["scores"].reshape(-1)
        base = c * EDGES_PER_CORE
        for r in range(NSUB * NSUB):
            sel, edge_off = positions[r]
            if len(sel):
                scores[base + sel] = core_out[edge_off:edge_off + len(sel)]
    return scores
